# revision 31
# baseline (speedup 1.0000x reference)
"""Multi-head attention (B=2, S=2048, D=1024, H=16) on 8 Trainium2 NeuronCores.

Sharding: batch x head-group. Core c handles batch b = c//4 and heads
[4*(c%4), 4*(c%4)+4) (a 256-wide slice of the QKV projection output and the
matching 256-row slice of Wo). Each core computes its partial output
projection; a 4-way ReduceScatter per batch group sums the partials and
writes each core's [128, 1024] row block of the final output directly, which
the host reassembles.

Per-core dataflow (all matmul operands fp16, fp32 PSUM accumulation):
  - x^T tiles arrive via hardware DMA-transpose (fp16), split across the
    two HWDGE queues (sync: K + half of V; scalar: weights, Q, rest of V)
    so the transposes run concurrently.
  - Q^T, K^T feature-major [256, 2048]; V token-major [2048, 256].
  - Scores computed transposed (S^T[k, q] = K_h @ Q_h^T) with zero-padded
    K^T per head (full-128 contraction, plain PE mode); softmax without
    max-subtraction (exp via ScalarE with the 1/sqrt(dh) scale folded in);
    per-q sums via a ones-matmul col-packed two heads at a time, broadcast
    across partitions for free; attn@V col-packed two heads at a time.
  - Emission order pipelines the first q-chunk's scores between Q proj and
    V proj so the PE keeps working while V's transposes stream in.
  - Output projection from attn_norm^T with the head-pair dim as the
    contraction tiles; per-q-chunk ReduceScatter writes out_d directly.
"""

import numpy as np

import concourse.bass as bass  # noqa: F401  (engine namespaces via nc)
import concourse.mybir as mybir
import concourse.tile as tile
from concourse import bacc
from concourse.bass import _add_dep_helper
from concourse.bass_utils import run_bass_kernel_spmd

F32 = mybir.dt.float32
F16 = mybir.dt.float16
AF = mybir.ActivationFunctionType

B, S, D = 2, 2048, 1024
H, DH = 16, 64
NCORES = 8
GPB = 4                # cores per batch group
HPC = H // GPB         # heads per core
DS = HPC * DH          # 256: per-core slice of the projection output
P = 128
NDT = D // P           # 8 d_model tiles
NTT = S // P           # 16 token tiles
QCH = 512              # q-chunk (PSUM bank = 512 fp32)
NQC = S // QCH         # 4
NKT = S // P           # 16 k tiles
SCALE = float(1.0 / np.sqrt(DH))

REPLICA_GROUPS = [[0, 1, 2, 3], [4, 5, 6, 7]]

_CACHED_NC = None


def _build_module():
    nc = bacc.Bacc("TRN2", target_bir_lowering=False, debug=False,
                   num_devices=NCORES)

    xq_d = nc.dram_tensor("xq", [S, D], F16, kind="ExternalInput")
    xk_d = nc.dram_tensor("xk", [S, D], F16, kind="ExternalInput")
    xv_d = nc.dram_tensor("xv", [S, D], F16, kind="ExternalInput")
    wq_d = nc.dram_tensor("wq", [D, DS], F16, kind="ExternalInput")
    wk_d = nc.dram_tensor("wk", [D, DS], F16, kind="ExternalInput")
    wv_d = nc.dram_tensor("wv", [D, DS], F16, kind="ExternalInput")
    wo_d = nc.dram_tensor("wo", [DS, D], F16, kind="ExternalInput")
    bq_d = nc.dram_tensor("bq", [DS, 1], F32, kind="ExternalInput")
    bk_d = nc.dram_tensor("bk", [DS, 1], F32, kind="ExternalInput")
    bv_d = nc.dram_tensor("bv", [1, DS], F32, kind="ExternalInput")
    bo_d = nc.dram_tensor("bo", [1, D], F32, kind="ExternalInput")

    out_d = nc.dram_tensor("out", [S // GPB, D], F16, kind="ExternalOutput")
    partial_cs = [nc.dram_tensor(f"partial{j}", [4 * P, D], F16)
                  for j in range(4)]
    rs_cs = [nc.dram_tensor(f"rs_out{j}", [P, D], F16)
             for j in range(4)]
    # last q-chunk: per-token-tile RS pieces (each core receives 32 rows)
    rs3_cs = [nc.dram_tensor(f"rs3_{t}", [P // 4, D], F16) for t in range(4)]

    with tile.TileContext(nc) as tc:
        with (
            tc.tile_pool(name="cst", bufs=1) as cst,
            tc.tile_pool(name="xt", bufs=14) as xtp,
            tc.tile_pool(name="exp", bufs=26) as expp,
            tc.tile_pool(name="rcp", bufs=4) as rcpp,
            tc.tile_pool(name="osb", bufs=3) as osbp,
            tc.tile_pool(name="psB", bufs=3, space="PSUM") as psB,
            tc.tile_pool(name="psC", bufs=1, space="PSUM") as psC,
        ):
            # Total PE ordering: chain every matmul to its predecessor
            # (nosync = scheduling-order only); on a PE-array tiling-mode
            # change (plain 128x128 <-> col-tiled via tile_position) add a
            # semaphore edge so the array drains before the mode flips --
            # a mode switch with an in-flight matmul corrupts results.
            _real_matmul = nc.tensor.matmul
            _prev_mm = {"inst": None, "mode": None}

            def mm(out, lhsT, rhs, **kw):
                mode = "col" if kw.get("tile_position") is not None else "plain"
                inst = _real_matmul(out, lhsT, rhs, **kw)
                if _prev_mm["inst"] is not None:
                    _add_dep_helper(
                        inst.ins, _prev_mm["inst"].ins,
                        sync=(mode != _prev_mm["mode"]),
                        reason="pe-mode-order")
                _prev_mm["inst"] = inst
                _prev_mm["mode"] = mode
                return inst

            # ---- constants (scalar HWDGE queue, wk first) ----
            wq_t = cst.tile([P, NDT, DS], F16, tag="wq")
            wk_t = cst.tile([P, NDT, DS], F16, tag="wk")
            wv_t = cst.tile([P, NDT, DS], F16, tag="wv")
            wo_t = cst.tile([P, 2, D], F16, tag="wo")
            bq_t = cst.tile([P, 2, 1], F32, tag="bq")
            bk_t = cst.tile([P, 2, 1], F32, tag="bk")
            bv_row = cst.tile([1, DS], F32, tag="bvr")
            bo_row = cst.tile([1, D], F32, tag="bor")

            # One DMA queue for everything: concurrent activity on the two
            # HWDGE queues ping-pongs the DMA xbar between transpose and
            # copy modes at ~8us per transition (and cross-queue transpose/
            # transpose corrupts data outright).
            nc.sync.dma_start(wk_t[:], wk_d.rearrange("(a p) n -> p a n", p=P))
            nc.sync.dma_start(wq_t[:], wq_d.rearrange("(a p) n -> p a n", p=P))
            nc.sync.dma_start(bk_t[:], bk_d.rearrange("(a p) o -> p a o", p=P))
            nc.sync.dma_start(bq_t[:], bq_d.rearrange("(a p) o -> p a o", p=P))
            nc.sync.dma_start(bv_row[:], bv_d[:])
            nc.sync.dma_start(bo_row[:], bo_d[:])
            nc.sync.dma_start(wv_t[:], wv_d.rearrange("(a p) n -> p a n", p=P))
            nc.sync.dma_start(wo_t[:], wo_d.rearrange("(a p) n -> p a n", p=P))

            bv_b = cst.tile([P, DS], F32, tag="bvb")
            bo_b = cst.tile([P, D], F32, tag="bob")
            nc.gpsimd.partition_broadcast(bv_b[:], bv_row[:])
            nc.gpsimd.partition_broadcast(bo_b[:], bo_row[:])

            ones_t = cst.tile([P, DH], F16, tag="ones")
            nc.vector.memset(ones_t[:], 1.0)

            # ---- activations: resident projections ----
            qt_t = cst.tile([P, 2, S], F16, tag="qt")   # Q^T  (pair, t)
            # K^T zero-padded per head: kz[:, h, :] has rows (h%2)*64..+64 =
            # K_h^T, other 64 rows zero -> full-K=128 scores matmuls at full
            # SBUF stream bandwidth, no PE row-tiling mode needed.
            kz_t = cst.tile([P, HPC, S], F16, tag="kz")
            nc.vector.memset(kz_t[:], 0.0)
            v_t = cst.tile([P, NTT, DS], F16, tag="vt")  # V token-major
            an_t = cst.tile([P, 2, S], F16, tag="an")   # attn_norm^T

            # ---- transposed input tiles (DMA transpose, fp16) ----
            def load_xt(x_d):
                tiles = []
                for dt in range(NDT):
                    t = xtp.tile([P, S], F16, tag="xt")
                    nc.sync.dma_start(
                        t[:], x_d[:, dt * P:(dt + 1) * P], transpose=True)
                    tiles.append(t)
                return tiles

            xt_k = load_xt(xk_d)
            xt_q = load_xt(xq_d)
            xt_v = load_xt(xv_d)

            # ---- feature-major projection: out^T[ds, t] (Q^T / K^T) ----
            # dt-interleaved across up to 3 q-chunks (3 PSUM tiles live) so
            # each transposed tile is consumed the moment it lands instead
            # of waiting for the full set.
            def proj_T_group(dst, w_t, b_t, xt, tcis):
                pss = {}
                for tci in tcis:
                    ps = psB.tile([P, 2 * QCH], F32, tag="sc", name=f"ps{tci}")
                    pss[tci] = ps
                for dt in range(NDT):
                    for tci in tcis:
                        ts0 = tci * QCH
                        for dot in range(2):
                            col = slice(dot * QCH, (dot + 1) * QCH)
                            mm(
                                pss[tci][:, col],
                                w_t[:, dt, dot * P:(dot + 1) * P],
                                xt[dt][:, ts0:ts0 + QCH],
                                start=(dt == 0), stop=(dt == NDT - 1),
                            )
                for tci in tcis:
                    ps = pss[tci]
                    ts0 = tci * QCH
                    if dst is qt_t:
                        for dot in range(2):
                            nc.scalar.activation(
                                dst[:, dot, ts0:ts0 + QCH],
                                ps[:, dot * QCH:(dot + 1) * QCH], AF.Identity,
                                bias=b_t[:, dot, :])
                    else:  # kz_t: per-head 64-row slices, rest stays zero
                        for h in range(HPC):
                            rows = slice((h % 2) * 64, (h % 2) * 64 + 64)
                            dot = h // 2
                            nc.scalar.activation(
                                kz_t[rows, h, ts0:ts0 + QCH],
                                ps[rows, dot * QCH:(dot + 1) * QCH],
                                AF.Identity,
                                bias=b_t[rows, dot, :])

            # ---- token-major V projection (one tile) ----
            def proj_v(tt):
                ps = psB.tile([P, DS], F32, tag="sc")
                for dt in range(NDT):
                    mm(
                        ps[:],
                        xt_v[dt][:, tt * P:(tt + 1) * P],
                        wv_t[:, dt, :],
                        start=(dt == 0), stop=(dt == NDT - 1),
                    )
                nc.vector.tensor_add(v_t[:, tt, :], ps[:], bv_b[:, :])

            # ---- attention phases ----
            # scores + exp for one kp step (2 k-blocks x 2 heads, plain)
            def ph1_kp(qc, pr, kp):
                qs = qc * QCH
                h0, h1 = 2 * pr, 2 * pr + 1
                sc0 = psB.tile([P, 2 * QCH], F32, tag="sc", name="sc0")
                sc1 = psB.tile([P, 2 * QCH], F32, tag="sc", name="sc1")
                for hsel, sc in ((h0, sc0), (h1, sc1)):
                    for j in range(2):
                        ks = (2 * kp + j) * P
                        col = slice(j * QCH, (j + 1) * QCH)
                        mm(
                            sc[:, col], kz_t[:, hsel, ks:ks + P],
                            qt_t[:, pr, qs:qs + QCH],
                            start=True, stop=True)
                e0 = expp.tile([P, 2 * QCH], F16, tag="exp", name="e0")
                e1 = expp.tile([P, 2 * QCH], F16, tag="exp", name="e1")
                nc.scalar.activation(e0[:], sc0[:], AF.Exp, scale=SCALE)
                nc.scalar.activation(e1[:], sc1[:], AF.Exp, scale=SCALE)
                return (e0, e1)

            # col-tiled attn@V + sums for a subset of k tiles, accumulating
            # into one acc/sm PSUM pair across the whole 16-kt sweep
            def ph2_chunk(qc, pr, etiles, kts, acc, sm):
                h0, h1 = 2 * pr, 2 * pr + 1
                for kt in kts:
                    e0, e1 = etiles[kt // 2]
                    col = slice((kt % 2) * QCH, (kt % 2 + 1) * QCH)
                    st = (kt == 0)
                    sp = (kt == NKT - 1)
                    mm(
                        sm[0:64, :], ones_t[:], e0[:, col],
                        start=st, stop=sp,
                        tile_position=(0, 0), skip_group_check=True)
                    mm(
                        sm[64:128, :], ones_t[:], e1[:, col],
                        start=st, stop=sp,
                        tile_position=(0, 64), skip_group_check=True)
                    mm(
                        acc[0:64, :], v_t[:, kt, h0 * DH:(h0 + 1) * DH],
                        e0[:, col], start=st, stop=sp,
                        tile_position=(0, 0), skip_group_check=True)
                    mm(
                        acc[64:128, :], v_t[:, kt, h1 * DH:(h1 + 1) * DH],
                        e1[:, col], start=st, stop=sp,
                        tile_position=(0, 64), skip_group_check=True)

            def norm(qc, pr, acc, sm):
                qs = qc * QCH
                rc = rcpp.tile([P, QCH], F32, tag="rcp", name="rc")
                nc.vector.reciprocal_approx_fast(rc[:], sm[:])
                nc.vector.tensor_mul(
                    an_t[:, pr, qs:qs + QCH], acc[:], rc[:])

            def po_tt(qc, tt4):
                """one token tile of the output projection (plain matmuls)."""
                tt = qc * (QCH // P) + tt4
                po = psB.tile([P, 2 * QCH], F32, tag="sc")
                for half in range(2):
                    for pr in range(2):
                        mm(
                            po[:, half * QCH:(half + 1) * QCH],
                            an_t[:, pr, tt * P:(tt + 1) * P],
                            wo_t[:, pr, half * QCH:(half + 1) * QCH],
                            start=(pr == 0), stop=(pr == 1))
                ob = osbp.tile([P, D], F16, tag="osb")
                nc.vector.tensor_add(ob[:], po[:], bo_b[:])
                nc.sync.dma_start(
                    partial_cs[tt // 4][(tt % 4) * P:(tt % 4 + 1) * P, :],
                    ob[:])

            def rs_qc(qc):
                """overlapped per-chunk ReduceScatter + out DMA."""
                nc.gpsimd.collective_compute(
                    "ReduceScatter",
                    mybir.AluOpType.add,
                    replica_groups=REPLICA_GROUPS,
                    ins=[partial_cs[qc][:]],
                    outs=[rs_cs[qc][:]],
                )
                nc.sync.dma_start(out_d[qc * P:(qc + 1) * P, :],
                                  rs_cs[qc][:])

            # ---- emission schedule ----
            proj_T_group(kz_t, wk_t, bk_t, xt_k, [0, 1, 2])
            proj_T_group(kz_t, wk_t, bk_t, xt_k, [3])
            proj_T_group(qt_t, wq_t, bq_t, xt_q, [0, 1, 2])
            proj_T_group(qt_t, wq_t, bq_t, xt_q, [3])

            # first q-chunk's pr0 scores run while V transposes stream in
            et00 = [ph1_kp(0, 0, kp) for kp in range(NKT // 2)]
            for tt in range(NTT):
                proj_v(tt)

            # software pipeline: each window runs ph1(cur) interleaved with
            # ph2(prev) at 2-kp granularity (the col-tiled attn@V fills the
            # PE's exp-paced PSUM-stall gaps; 8 mode switches per window).
            # pr1 windows also carry the output projection of q-chunk qc-1
            # (whose an_t completed at the previous window's end) + its RS.
            prev = (0, 0, et00)
            seq = [(qc, pr) for qc in range(NQC) for pr in range(2)][1:]
            for qc, pr in seq:
                pacc = psC.tile([P, QCH], F32, tag="acc", name="pacc")
                psm = psC.tile([P, QCH], F32, tag="sum", name="psm")
                po_src = qc - 1 if (pr == 1 and qc >= 1) else None
                et = []
                for kpp in range(NKT // 4):
                    et.append(ph1_kp(qc, pr, 2 * kpp))
                    et.append(ph1_kp(qc, pr, 2 * kpp + 1))
                    ph2_chunk(prev[0], prev[1], prev[2],
                              [4 * kpp + i for i in range(4)], pacc, psm)
                    if po_src is not None:
                        po_tt(po_src, kpp)
                if po_src is not None:
                    rs_qc(po_src)
                norm(prev[0], prev[1], pacc, psm)
                prev = (qc, pr, et)

            lacc = psC.tile([P, QCH], F32, tag="acc", name="lacc")
            lsm = psC.tile([P, QCH], F32, tag="sum", name="lsm")
            ph2_chunk(prev[0], prev[1], prev[2], list(range(NKT)), lacc, lsm)
            norm(prev[0], prev[1], lacc, lsm)
            # last q-chunk: fire a small RS per token tile the moment its
            # partial lands, so the cross-core waits pipeline instead of
            # stacking after the full 512-row block
            for tt4 in range(4):
                po_tt(3, tt4)
                nc.gpsimd.collective_compute(
                    "ReduceScatter",
                    mybir.AluOpType.add,
                    replica_groups=REPLICA_GROUPS,
                    ins=[partial_cs[3][tt4 * P:(tt4 + 1) * P, :]],
                    outs=[rs3_cs[tt4][:]],
                )
                nc.sync.dma_start(
                    out_d[3 * P + tt4 * 32:3 * P + tt4 * 32 + 32, :],
                    rs3_cs[tt4][:])

    nc.compile()
    return nc


def _get_nc():
    global _CACHED_NC
    if _CACHED_NC is None:
        _CACHED_NC = _build_module()
    return _CACHED_NC


def _swap_pairs_rows(wo_slice):
    """Swap the two 64-row head blocks within each head pair (kernel's
    phase-2 PSUM layout has h1 in partitions 0-63)."""
    out = wo_slice.copy()
    for pr in range(2):
        a = pr * 2 * DH
        out[a:a + DH], out[a + DH:a + 2 * DH] = \
            wo_slice[a + DH:a + 2 * DH].copy(), wo_slice[a:a + DH].copy()
    return out


def _make_in_maps(query, key, value, Wq, bq, Wk, bk, Wv, bv, Wo, bo):
    query = np.asarray(query, dtype=np.float32)
    key = np.asarray(key, dtype=np.float32)
    value = np.asarray(value, dtype=np.float32)
    Wq = np.asarray(Wq, dtype=np.float32)
    Wk = np.asarray(Wk, dtype=np.float32)
    Wv = np.asarray(Wv, dtype=np.float32)
    Wo = np.asarray(Wo, dtype=np.float32)
    bq = np.asarray(bq, dtype=np.float32)
    bk = np.asarray(bk, dtype=np.float32)
    bv = np.asarray(bv, dtype=np.float32)
    bo = np.asarray(bo, dtype=np.float32)

    in_maps = []
    for c in range(NCORES):
        b = c // GPB
        g = c % GPB
        sl = slice(g * DS, (g + 1) * DS)
        in_maps.append({
            "xq": query[b].astype(np.float16),
            "xk": key[b].astype(np.float16),
            "xv": value[b].astype(np.float16),
            "wq": Wq[:, sl].astype(np.float16),
            "wk": Wk[:, sl].astype(np.float16),
            "wv": Wv[:, sl].astype(np.float16),
            "wo": Wo[sl, :].astype(np.float16),
            "bq": bq[sl].reshape(DS, 1).copy(),
            "bk": bk[sl].reshape(DS, 1).copy(),
            "bv": bv[sl].reshape(1, DS).copy(),
            "bo": (bo if g == 0 else np.zeros_like(bo)).reshape(1, D).copy(),
        })
    return in_maps


def run(inputs, trace=False, trace_cores=None):
    """Run the SPMD kernel; returns (full_output, BassKernelResults)."""
    nc = _get_nc()
    in_maps = _make_in_maps(**inputs)
    res = run_bass_kernel_spmd(
        nc, in_maps, core_ids=list(range(NCORES)), trace=trace,
        trace_cores=trace_cores)
    out = np.empty((B, S, D), dtype=np.float32)
    for c in range(NCORES):
        b = c // GPB
        g = c % GPB
        o = res.results[c]["out"].astype(np.float32)
        for j in range(3):
            out[b, j * 512 + g * P:j * 512 + (g + 1) * P, :] = \
                o[j * P:(j + 1) * P, :]
        for t in range(4):
            r0 = 3 * 512 + t * P + g * 32
            out[b, r0:r0 + 32, :] = o[3 * P + t * 32:3 * P + (t + 1) * 32, :]
    return out, res


def kernel(**inputs):
    out, _ = run(inputs, trace=False)
    return out


# revision 32
# speedup vs baseline: 1.0961x; 1.0961x over previous
"""Multi-head attention (B=2, S=2048, D=1024, H=16) on 8 Trainium2 NeuronCores.

Sharding: batch x head-group. Core c handles batch b = c//4 and heads
[4*(c%4), 4*(c%4)+4) (a 256-wide slice of the QKV projection output and the
matching 256-row slice of Wo). Each core computes its partial output
projection; a 4-way ReduceScatter per batch group sums the partials and
writes each core's [128, 1024] row block of the final output directly, which
the host reassembles.

Per-core dataflow (all matmul operands fp16, fp32 PSUM accumulation):
  - x^T tiles arrive via hardware DMA-transpose (fp16), split across the
    two HWDGE queues (sync: K + half of V; scalar: weights, Q, rest of V)
    so the transposes run concurrently.
  - Q^T, K^T feature-major [256, 2048]; V token-major [2048, 256].
  - Scores computed transposed (S^T[k, q] = K_h @ Q_h^T) with zero-padded
    K^T per head (full-128 contraction, plain PE mode); softmax without
    max-subtraction (exp via ScalarE with the 1/sqrt(dh) scale folded in);
    per-q sums via a ones-matmul col-packed two heads at a time, broadcast
    across partitions for free; attn@V col-packed two heads at a time.
  - Emission order pipelines the first q-chunk's scores between Q proj and
    V proj so the PE keeps working while V's transposes stream in.
  - Output projection from attn_norm^T with the head-pair dim as the
    contraction tiles; per-q-chunk ReduceScatter writes out_d directly.
"""

import numpy as np

import concourse.bass as bass  # noqa: F401  (engine namespaces via nc)
import concourse.mybir as mybir
import concourse.tile as tile
from concourse import bacc
from concourse.bass import _add_dep_helper
from concourse.bass_utils import run_bass_kernel_spmd

F32 = mybir.dt.float32
F16 = mybir.dt.float16
AF = mybir.ActivationFunctionType

B, S, D = 2, 2048, 1024
H, DH = 16, 64
NCORES = 8
GPB = 4                # cores per batch group
HPC = H // GPB         # heads per core
DS = HPC * DH          # 256: per-core slice of the projection output
P = 128
NDT = D // P           # 8 d_model tiles
NTT = S // P           # 16 token tiles
QCH = 512              # q-chunk (PSUM bank = 512 fp32)
NQC = S // QCH         # 4
NKT = S // P           # 16 k tiles
SCALE = float(1.0 / np.sqrt(DH))

REPLICA_GROUPS = [[0, 1, 2, 3], [4, 5, 6, 7]]

_CACHED_NC = None


def _build_module():
    nc = bacc.Bacc("TRN2", target_bir_lowering=False, debug=False,
                   num_devices=NCORES)

    xq_d = nc.dram_tensor("xq", [S, D], F16, kind="ExternalInput")
    xk_d = nc.dram_tensor("xk", [S, D], F16, kind="ExternalInput")
    xv_d = nc.dram_tensor("xv", [S, D], F16, kind="ExternalInput")
    wq_d = nc.dram_tensor("wq", [D, DS], F16, kind="ExternalInput")
    wk_d = nc.dram_tensor("wk", [D, DS], F16, kind="ExternalInput")
    wv_d = nc.dram_tensor("wv", [D, DS], F16, kind="ExternalInput")
    wo_d = nc.dram_tensor("wo", [DS, D], F16, kind="ExternalInput")
    bq_d = nc.dram_tensor("bq", [DS, 1], F32, kind="ExternalInput")
    bk_d = nc.dram_tensor("bk", [DS, 1], F32, kind="ExternalInput")
    bv_d = nc.dram_tensor("bv", [1, DS], F32, kind="ExternalInput")
    bo_d = nc.dram_tensor("bo", [1, D], F32, kind="ExternalInput")

    out_d = nc.dram_tensor("out", [S // GPB, D], F16, kind="ExternalOutput")
    partial_cs = [nc.dram_tensor(f"partial{j}", [4 * P, D], F16)
                  for j in range(4)]
    rs_cs = [nc.dram_tensor(f"rs_out{j}", [P, D], F16)
             for j in range(4)]

    with tile.TileContext(nc) as tc:
        with (
            tc.tile_pool(name="cst", bufs=1) as cst,
            tc.tile_pool(name="xt", bufs=14) as xtp,
            tc.tile_pool(name="exp", bufs=26) as expp,
            tc.tile_pool(name="rcp", bufs=4) as rcpp,
            tc.tile_pool(name="osb", bufs=3) as osbp,
            tc.tile_pool(name="psB", bufs=3, space="PSUM") as psB,
            tc.tile_pool(name="psC", bufs=1, space="PSUM") as psC,
        ):
            # Total PE ordering: chain every matmul to its predecessor
            # (nosync = scheduling-order only); on a PE-array tiling-mode
            # change (plain 128x128 <-> col-tiled via tile_position) add a
            # semaphore edge so the array drains before the mode flips --
            # a mode switch with an in-flight matmul corrupts results.
            _real_matmul = nc.tensor.matmul
            _prev_mm = {"inst": None, "mode": None}

            def mm(out, lhsT, rhs, **kw):
                mode = "col" if kw.get("tile_position") is not None else "plain"
                inst = _real_matmul(out, lhsT, rhs, **kw)
                if _prev_mm["inst"] is not None:
                    _add_dep_helper(
                        inst.ins, _prev_mm["inst"].ins,
                        sync=(mode != _prev_mm["mode"]),
                        reason="pe-mode-order")
                _prev_mm["inst"] = inst
                _prev_mm["mode"] = mode
                return inst

            # ---- constants (scalar HWDGE queue, wk first) ----
            wq_t = cst.tile([P, NDT, DS], F16, tag="wq")
            wk_t = cst.tile([P, NDT, DS], F16, tag="wk")
            wv_t = cst.tile([P, NDT, DS], F16, tag="wv")
            wo_t = cst.tile([P, 2, D], F16, tag="wo")
            bq_t = cst.tile([P, 2, 1], F32, tag="bq")
            bk_t = cst.tile([P, 2, 1], F32, tag="bk")
            bv_row = cst.tile([1, DS], F32, tag="bvr")
            bo_row = cst.tile([1, D], F32, tag="bor")

            # One DMA queue for everything: concurrent activity on the two
            # HWDGE queues ping-pongs the DMA xbar between transpose and
            # copy modes at ~8us per transition (and cross-queue transpose/
            # transpose corrupts data outright).
            nc.sync.dma_start(wk_t[:], wk_d.rearrange("(a p) n -> p a n", p=P))
            nc.sync.dma_start(wq_t[:], wq_d.rearrange("(a p) n -> p a n", p=P))
            nc.sync.dma_start(bk_t[:], bk_d.rearrange("(a p) o -> p a o", p=P))
            nc.sync.dma_start(bq_t[:], bq_d.rearrange("(a p) o -> p a o", p=P))
            nc.sync.dma_start(bv_row[:], bv_d[:])
            nc.sync.dma_start(bo_row[:], bo_d[:])
            nc.sync.dma_start(wv_t[:], wv_d.rearrange("(a p) n -> p a n", p=P))
            nc.sync.dma_start(wo_t[:], wo_d.rearrange("(a p) n -> p a n", p=P))

            bv_b = cst.tile([P, DS], F32, tag="bvb")
            bo_b = cst.tile([P, D], F32, tag="bob")
            nc.gpsimd.partition_broadcast(bv_b[:], bv_row[:])
            nc.gpsimd.partition_broadcast(bo_b[:], bo_row[:])

            ones_t = cst.tile([P, DH], F16, tag="ones")
            nc.vector.memset(ones_t[:], 1.0)

            # ---- activations: resident projections ----
            qt_t = cst.tile([P, 2, S], F16, tag="qt")   # Q^T  (pair, t)
            # K^T zero-padded per head: kz[:, h, :] has rows (h%2)*64..+64 =
            # K_h^T, other 64 rows zero -> full-K=128 scores matmuls at full
            # SBUF stream bandwidth, no PE row-tiling mode needed.
            kz_t = cst.tile([P, HPC, S], F16, tag="kz")
            nc.vector.memset(kz_t[:], 0.0)
            v_t = cst.tile([P, NTT, DS], F16, tag="vt")  # V token-major
            an_t = cst.tile([P, 2, S], F16, tag="an")   # attn_norm^T

            # ---- transposed input tiles (DMA transpose, fp16) ----
            def load_xt(x_d):
                tiles = []
                for dt in range(NDT):
                    t = xtp.tile([P, S], F16, tag="xt")
                    nc.sync.dma_start(
                        t[:], x_d[:, dt * P:(dt + 1) * P], transpose=True)
                    tiles.append(t)
                return tiles

            xt_k = load_xt(xk_d)
            xt_q = load_xt(xq_d)
            xt_v = load_xt(xv_d)

            # ---- feature-major projection: out^T[ds, t] (Q^T / K^T) ----
            # dt-interleaved across up to 3 q-chunks (3 PSUM tiles live) so
            # each transposed tile is consumed the moment it lands instead
            # of waiting for the full set.
            def proj_T_group(dst, w_t, b_t, xt, tcis):
                pss = {}
                for tci in tcis:
                    ps = psB.tile([P, 2 * QCH], F32, tag="sc", name=f"ps{tci}")
                    pss[tci] = ps
                for dt in range(NDT):
                    for tci in tcis:
                        ts0 = tci * QCH
                        for dot in range(2):
                            col = slice(dot * QCH, (dot + 1) * QCH)
                            mm(
                                pss[tci][:, col],
                                w_t[:, dt, dot * P:(dot + 1) * P],
                                xt[dt][:, ts0:ts0 + QCH],
                                start=(dt == 0), stop=(dt == NDT - 1),
                            )
                for tci in tcis:
                    ps = pss[tci]
                    ts0 = tci * QCH
                    if dst is qt_t:
                        for dot in range(2):
                            nc.scalar.activation(
                                dst[:, dot, ts0:ts0 + QCH],
                                ps[:, dot * QCH:(dot + 1) * QCH], AF.Identity,
                                bias=b_t[:, dot, :])
                    else:  # kz_t: per-head 64-row slices, rest stays zero
                        for h in range(HPC):
                            rows = slice((h % 2) * 64, (h % 2) * 64 + 64)
                            dot = h // 2
                            nc.scalar.activation(
                                kz_t[rows, h, ts0:ts0 + QCH],
                                ps[rows, dot * QCH:(dot + 1) * QCH],
                                AF.Identity,
                                bias=b_t[rows, dot, :])

            # ---- token-major V projection (one tile) ----
            def proj_v(tt):
                ps = psB.tile([P, DS], F32, tag="sc")
                for dt in range(NDT):
                    mm(
                        ps[:],
                        xt_v[dt][:, tt * P:(tt + 1) * P],
                        wv_t[:, dt, :],
                        start=(dt == 0), stop=(dt == NDT - 1),
                    )
                nc.vector.tensor_add(v_t[:, tt, :], ps[:], bv_b[:, :])

            # ---- attention phases ----
            # scores + exp for one kp step (2 k-blocks x 2 heads, plain)
            def ph1_kp(qc, pr, kp):
                qs = qc * QCH
                h0, h1 = 2 * pr, 2 * pr + 1
                sc0 = psB.tile([P, 2 * QCH], F32, tag="sc", name="sc0")
                sc1 = psB.tile([P, 2 * QCH], F32, tag="sc", name="sc1")
                for hsel, sc in ((h0, sc0), (h1, sc1)):
                    for j in range(2):
                        ks = (2 * kp + j) * P
                        col = slice(j * QCH, (j + 1) * QCH)
                        mm(
                            sc[:, col], kz_t[:, hsel, ks:ks + P],
                            qt_t[:, pr, qs:qs + QCH],
                            start=True, stop=True)
                e0 = expp.tile([P, 2 * QCH], F16, tag="exp", name="e0")
                e1 = expp.tile([P, 2 * QCH], F16, tag="exp", name="e1")
                nc.scalar.activation(e0[:], sc0[:], AF.Exp, scale=SCALE)
                nc.scalar.activation(e1[:], sc1[:], AF.Exp, scale=SCALE)
                return (e0, e1)

            # col-tiled attn@V + sums for a subset of k tiles, accumulating
            # into one acc/sm PSUM pair across the whole 16-kt sweep
            def ph2_chunk(qc, pr, etiles, kts, acc, sm):
                h0, h1 = 2 * pr, 2 * pr + 1
                for kt in kts:
                    e0, e1 = etiles[kt // 2]
                    col = slice((kt % 2) * QCH, (kt % 2 + 1) * QCH)
                    st = (kt == 0)
                    sp = (kt == NKT - 1)
                    mm(
                        sm[0:64, :], ones_t[:], e0[:, col],
                        start=st, stop=sp,
                        tile_position=(0, 0), skip_group_check=True)
                    mm(
                        sm[64:128, :], ones_t[:], e1[:, col],
                        start=st, stop=sp,
                        tile_position=(0, 64), skip_group_check=True)
                    mm(
                        acc[0:64, :], v_t[:, kt, h0 * DH:(h0 + 1) * DH],
                        e0[:, col], start=st, stop=sp,
                        tile_position=(0, 0), skip_group_check=True)
                    mm(
                        acc[64:128, :], v_t[:, kt, h1 * DH:(h1 + 1) * DH],
                        e1[:, col], start=st, stop=sp,
                        tile_position=(0, 64), skip_group_check=True)

            def norm(qc, pr, acc, sm):
                qs = qc * QCH
                rc = rcpp.tile([P, QCH], F32, tag="rcp", name="rc")
                nc.vector.reciprocal_approx_fast(rc[:], sm[:])
                nc.vector.tensor_mul(
                    an_t[:, pr, qs:qs + QCH], acc[:], rc[:])

            def po_tt(qc, tt4):
                """one token tile of the output projection (plain matmuls)."""
                tt = qc * (QCH // P) + tt4
                po = psB.tile([P, 2 * QCH], F32, tag="sc")
                for half in range(2):
                    for pr in range(2):
                        mm(
                            po[:, half * QCH:(half + 1) * QCH],
                            an_t[:, pr, tt * P:(tt + 1) * P],
                            wo_t[:, pr, half * QCH:(half + 1) * QCH],
                            start=(pr == 0), stop=(pr == 1))
                ob = osbp.tile([P, D], F16, tag="osb")
                nc.vector.tensor_add(ob[:], po[:], bo_b[:])
                nc.sync.dma_start(
                    partial_cs[tt // 4][(tt % 4) * P:(tt % 4 + 1) * P, :],
                    ob[:])

            def rs_qc(qc):
                """overlapped per-chunk ReduceScatter + out DMA."""
                nc.gpsimd.collective_compute(
                    "ReduceScatter",
                    mybir.AluOpType.add,
                    replica_groups=REPLICA_GROUPS,
                    ins=[partial_cs[qc][:]],
                    outs=[rs_cs[qc][:]],
                )
                nc.sync.dma_start(out_d[qc * P:(qc + 1) * P, :],
                                  rs_cs[qc][:])

            # ---- emission schedule ----
            proj_T_group(kz_t, wk_t, bk_t, xt_k, [0, 1, 2])
            proj_T_group(kz_t, wk_t, bk_t, xt_k, [3])
            proj_T_group(qt_t, wq_t, bq_t, xt_q, [0, 1, 2])
            proj_T_group(qt_t, wq_t, bq_t, xt_q, [3])

            # first q-chunk's pr0 scores run while V transposes stream in
            et00 = [ph1_kp(0, 0, kp) for kp in range(NKT // 2)]
            for tt in range(NTT):
                proj_v(tt)

            # software pipeline: each window runs ph1(cur) interleaved with
            # ph2(prev) at 2-kp granularity (the col-tiled attn@V fills the
            # PE's exp-paced PSUM-stall gaps; 8 mode switches per window).
            # pr1 windows also carry the output projection of q-chunk qc-1
            # (whose an_t completed at the previous window's end) + its RS.
            prev = (0, 0, et00)
            seq = [(qc, pr) for qc in range(NQC) for pr in range(2)][1:]
            for qc, pr in seq:
                pacc = psC.tile([P, QCH], F32, tag="acc", name="pacc")
                psm = psC.tile([P, QCH], F32, tag="sum", name="psm")
                po_src = qc - 1 if (pr == 1 and qc >= 1) else None
                et = []
                for kpp in range(NKT // 4):
                    et.append(ph1_kp(qc, pr, 2 * kpp))
                    et.append(ph1_kp(qc, pr, 2 * kpp + 1))
                    ph2_chunk(prev[0], prev[1], prev[2],
                              [4 * kpp + i for i in range(4)], pacc, psm)
                    if po_src is not None:
                        po_tt(po_src, kpp)
                if po_src is not None:
                    rs_qc(po_src)
                norm(prev[0], prev[1], pacc, psm)
                prev = (qc, pr, et)

            lacc = psC.tile([P, QCH], F32, tag="acc", name="lacc")
            lsm = psC.tile([P, QCH], F32, tag="sum", name="lsm")
            ph2_chunk(prev[0], prev[1], prev[2], list(range(NKT)), lacc, lsm)
            norm(prev[0], prev[1], lacc, lsm)
            po_tt(3, 0)
            po_tt(3, 1)
            po_tt(3, 2)
            po_tt(3, 3)
            rs_qc(3)

    nc.compile()
    return nc


def _get_nc():
    global _CACHED_NC
    if _CACHED_NC is None:
        _CACHED_NC = _build_module()
    return _CACHED_NC


def _swap_pairs_rows(wo_slice):
    """Swap the two 64-row head blocks within each head pair (kernel's
    phase-2 PSUM layout has h1 in partitions 0-63)."""
    out = wo_slice.copy()
    for pr in range(2):
        a = pr * 2 * DH
        out[a:a + DH], out[a + DH:a + 2 * DH] = \
            wo_slice[a + DH:a + 2 * DH].copy(), wo_slice[a:a + DH].copy()
    return out


def _make_in_maps(query, key, value, Wq, bq, Wk, bk, Wv, bv, Wo, bo):
    query = np.asarray(query, dtype=np.float32)
    key = np.asarray(key, dtype=np.float32)
    value = np.asarray(value, dtype=np.float32)
    Wq = np.asarray(Wq, dtype=np.float32)
    Wk = np.asarray(Wk, dtype=np.float32)
    Wv = np.asarray(Wv, dtype=np.float32)
    Wo = np.asarray(Wo, dtype=np.float32)
    bq = np.asarray(bq, dtype=np.float32)
    bk = np.asarray(bk, dtype=np.float32)
    bv = np.asarray(bv, dtype=np.float32)
    bo = np.asarray(bo, dtype=np.float32)

    in_maps = []
    for c in range(NCORES):
        b = c // GPB
        g = c % GPB
        sl = slice(g * DS, (g + 1) * DS)
        in_maps.append({
            "xq": query[b].astype(np.float16),
            "xk": key[b].astype(np.float16),
            "xv": value[b].astype(np.float16),
            "wq": Wq[:, sl].astype(np.float16),
            "wk": Wk[:, sl].astype(np.float16),
            "wv": Wv[:, sl].astype(np.float16),
            "wo": Wo[sl, :].astype(np.float16),
            "bq": bq[sl].reshape(DS, 1).copy(),
            "bk": bk[sl].reshape(DS, 1).copy(),
            "bv": bv[sl].reshape(1, DS).copy(),
            "bo": (bo if g == 0 else np.zeros_like(bo)).reshape(1, D).copy(),
        })
    return in_maps


def run(inputs, trace=False, trace_cores=None):
    """Run the SPMD kernel; returns (full_output, BassKernelResults)."""
    nc = _get_nc()
    in_maps = _make_in_maps(**inputs)
    res = run_bass_kernel_spmd(
        nc, in_maps, core_ids=list(range(NCORES)), trace=trace,
        trace_cores=trace_cores)
    out = np.empty((B, S, D), dtype=np.float32)
    for c in range(NCORES):
        b = c // GPB
        g = c % GPB
        o = res.results[c]["out"].astype(np.float32)
        for j in range(4):
            out[b, j * 512 + g * P:j * 512 + (g + 1) * P, :] = \
                o[j * P:(j + 1) * P, :]
    return out, res


def kernel(**inputs):
    out, _ = run(inputs, trace=False)
    return out
